# revision 4
# baseline (speedup 1.0000x reference)
"""Trainium2 Bass kernel for nn_CopiedSetEncoder (set encoder with recurrent
attention). Self-contained: shards batch across 8 NeuronCores, builds a
length-specialized SPMD Tile kernel, runs it, and reassembles the output.

v2 design notes (vs baseline):
- Contiguous token packing (no per-slot 128-chunk alignment): t_common drops
  ~13%. Slot ownership is encoded purely in the mask / weight tiles.
- embB (token-major embeddings) produced by PE transposes of embA instead of
  a second bank of W3 matmuls: 4096 -> 1024 PE rows per 512-token tile.
- Softmax uses unnormalized f16 weights exp(logit - C1) with C1=4 (logits on
  this problem are ~|0.2|), and a ones-column appended to embB so one matmul
  chain yields both attended and the normalizer S. Normalization happens on
  the [16, 257] result. This kills the per-iteration sum-matmul chain (68
  matmuls), reciprocal broadcast transposes, and the big normalize multiply.
- LSTM sigmoids computed as 0.5*(1+tanh(x/2)): exp/tanh/relu/copy share one
  activation table set, so the Act engine never swaps tables (saves ~1.3us
  per swap, 2 swaps/iteration in the baseline).
"""
import numpy as np

import concourse.bass as bass
import concourse.mybir as mybir
import concourse.tile as tile
from concourse.bass_utils import run_bass_kernel_spmd

B, F_, D_IN = 128, 1024, 128
H1, H2, E, H = 512, 512, 256, 256
N_SHUFFLE = 5
NCORES = 8
BLOC = B // NCORES  # 16 batches per core
NEG = -1e30
C1 = 4.0  # logit shift for max-free softmax (logits here are tiny)

f32 = mybir.dt.float32
f16 = mybir.dt.float16


def _split_multi_waits(nc):
    """HW allows at most one sync wait per instruction; hoist extras into
    standalone InstEventSemaphore carriers on the same engine."""
    cnt = 0
    for bb in nc.main_func.blocks:
        insts = bb.instructions  # live list
        i = 0
        while i < len(insts):
            ins = insts[i]
            si = ins.sync_info
            if si is not None and si.on_wait and len(si.on_wait) > 1:
                waits = list(si.on_wait)
                carriers = []
                for w in waits[:-1]:
                    cnt += 1
                    ev = mybir.InstEventSemaphore(name=f"wsplit-{cnt}")
                    ev.engine = ins.engine
                    ev.sync_info = mybir.SyncInfo(on_wait=[w], on_update=[])
                    carriers.append(ev)
                ins.sync_info = mybir.SyncInfo(
                    on_wait=[waits[-1]], on_update=list(si.on_update)
                )
                for j, ev in enumerate(carriers):
                    insts.insert(i + j, ev)
                    nc.register_instruction(ev, overwrite=True)
                i += len(carriers)
            i += 1
    return cnt


def _build_module(t_common):
    """One SPMD program for all cores. Tokens packed contiguously per core;
    slot membership is data (mask/w0T), not structure."""
    nc = bass.Bass()
    n_tiles = t_common // 512
    tot_chunks = t_common // 128

    # ---- inputs ----
    xT_e = nc.declare_dram_parameter("xT", [128, t_common], f16, isOutput=False)
    w1_e = nc.declare_dram_parameter("w1", [128, H1], f16, isOutput=False)
    w2_e = nc.declare_dram_parameter("w2", [128, 4, H2], f16, isOutput=False)
    w3_e = nc.declare_dram_parameter("w3", [128, 4, E], f16, isOutput=False)
    wih_e = nc.declare_dram_parameter("wih", [128, 2, 4 * H], f16, isOutput=False)
    whh_e = nc.declare_dram_parameter("whh", [128, 2, 4 * H], f16, isOutput=False)
    b1_e = nc.declare_dram_parameter("b1", [128, 4], f32, isOutput=False)
    b2_e = nc.declare_dram_parameter("b2", [128, 4], f32, isOutput=False)
    bg_e = nc.declare_dram_parameter("bg", [128, 8], f32, isOutput=False)
    mask_e = nc.declare_dram_parameter(
        "mask", [128, tot_chunks, BLOC], f32, isOutput=False
    )
    w0T_e = nc.declare_dram_parameter(
        "w0T", [128, tot_chunks, BLOC], f16, isOutput=False
    )
    ident_e = nc.declare_dram_parameter("ident", [128, 128], f32, isOutput=False)
    ident16_e = nc.declare_dram_parameter("ident16", [128, 128], f16, isOutput=False)
    att_o = nc.declare_dram_parameter("att", [BLOC, E], f32, isOutput=True)
    qt_o = nc.declare_dram_parameter("qt", [BLOC, H], f32, isOutput=True)

    with tile.TileContext(nc) as tc:
        with tc.tile_pool(name="big", bufs=1) as big, \
             tc.tile_pool(name="wp", bufs=1) as wp:
            # resident tensors
            xT = big.tile([128, t_common], f16)
            embA = big.tile([128, 2, t_common], f16)
            embB = big.tile([128, tot_chunks, E + 1], f16)
            w1 = wp.tile([128, H1], f16)
            w2 = wp.tile([128, 4, H2], f16)
            w3 = wp.tile([128, 4, E], f16)
            wih = wp.tile([128, 2, 4 * H], f16)
            whh = wp.tile([128, 2, 4 * H], f16)
            b1 = wp.tile([128, 4], f32)
            b2 = wp.tile([128, 4], f32)
            bg = wp.tile([128, 8], f32)
            mask = wp.tile([128, tot_chunks, BLOC], f32)
            w0T = wp.tile([128, tot_chunks, BLOC], f16)
            ident = wp.tile([128, 128], f32)
            ident16 = wp.tile([128, 128], f16)
            for dst, src in [
                (w1, w1_e), (w2, w2_e), (w3, w3_e),
                (wih, wih_e), (whh, whh_e), (b1, b1_e), (b2, b2_e),
                (bg, bg_e), (mask, mask_e), (w0T, w0T_e), (ident, ident_e),
                (ident16, ident16_e),
            ]:
                nc.sync.dma_start(out=dst[:], in_=src[:])
            # xT in pieces so tile 0 compute starts before the full upload
            xq = t_common // 4  # t_common is a multiple of 2048
            for q in range(4):
                nc.sync.dma_start(
                    out=xT[:, q * xq:(q + 1) * xq], in_=xT_e[:, q * xq:(q + 1) * xq]
                )
            # ones column of embB (col E), once
            nc.vector.memset(embB[:, :, E:E + 1], 1.0)

            # ---- phase 1: MLP over 512-token tiles; embB via PE transpose ----
            with tc.tile_pool(name="mlp", bufs=3) as mp, \
                 tc.tile_pool(name="ps1", bufs=2, space="PSUM") as ps1, \
                 tc.tile_pool(name="ps2", bufs=2, space="PSUM") as ps2, \
                 tc.tile_pool(name="ps3", bufs=2, space="PSUM") as ps3, \
                 tc.tile_pool(name="psT", bufs=2, space="PSUM") as psT:
                for t in range(n_tiles):
                    sl = slice(t * 512, (t + 1) * 512)
                    h1t = mp.tile([128, 4, 512], f16, tag="h1")
                    for mc in range(4):
                        p = ps1.tile([128, 512], f32, tag="pA")
                        nc.tensor.matmul(
                            p[:], w1[:, mc * 128:(mc + 1) * 128], xT[:, sl],
                            start=True, stop=True,
                        )
                        if mc % 2 == 0:
                            nc.scalar.activation(
                                out=h1t[:, mc, :], in_=p[:],
                                func=mybir.ActivationFunctionType.Relu,
                                bias=b1[:, mc:mc + 1], scale=1.0,
                            )
                        else:
                            nc.vector.tensor_scalar(
                                out=h1t[:, mc, :], in0=p[:], scalar1=b1[:, mc:mc + 1],
                                scalar2=0.0, op0=mybir.AluOpType.add,
                                op1=mybir.AluOpType.max,
                            )
                    h2t = mp.tile([128, 4, 512], f16, tag="h2")
                    for mc in range(4):
                        p = ps2.tile([128, 512], f32, tag="pB")
                        for kc in range(4):
                            nc.tensor.matmul(
                                p[:], w2[:, kc, mc * 128:(mc + 1) * 128],
                                h1t[:, kc, :], start=(kc == 0), stop=(kc == 3),
                            )
                        if mc % 2 == 0:
                            nc.scalar.activation(
                                out=h2t[:, mc, :], in_=p[:],
                                func=mybir.ActivationFunctionType.Relu,
                                bias=b2[:, mc:mc + 1], scale=1.0,
                            )
                        else:
                            nc.vector.tensor_scalar(
                                out=h2t[:, mc, :], in0=p[:], scalar1=b2[:, mc:mc + 1],
                                scalar2=0.0, op0=mybir.AluOpType.add,
                                op1=mybir.AluOpType.max,
                            )
                    # embA: [e-chunk partitions, tokens]
                    for mc in range(2):
                        p = ps3.tile([128, 512], f32, tag="pC")
                        for kc in range(4):
                            nc.tensor.matmul(
                                p[:], w3[:, kc, mc * 128:(mc + 1) * 128],
                                h2t[:, kc, :], start=(kc == 0), stop=(kc == 3),
                            )
                        if mc == 0:
                            nc.scalar.copy(out=embA[:, mc, sl], in_=p[:])
                        else:
                            nc.vector.tensor_copy(embA[:, mc, sl], p[:])
                    # embB: transpose embA 128-token blocks -> [tok, E]
                    for s in range(4):
                        ch = t * 4 + s
                        tsl = slice(t * 512 + s * 128, t * 512 + (s + 1) * 128)
                        pt = psT.tile([128, 2, 128], f16, tag="pT")
                        for kc in range(2):
                            nc.tensor.transpose(
                                pt[:, kc, :], embA[:, kc, tsl], ident16[:, :]
                            )
                        if s % 2 == 0:
                            nc.scalar.copy(out=embB[:, ch, :E], in_=pt[:])
                        else:
                            nc.vector.tensor_copy(embB[:, ch, :E], pt[:])

            # ---- phase 2: recurrent attention ----
            n_grp = (tot_chunks + 7) // 8
            with tc.tile_pool(name="att", bufs=1) as ap, \
                 tc.tile_pool(name="attd", bufs=2) as ad, \
                 tc.tile_pool(name="psL", bufs=2, space="PSUM") as psL, \
                 tc.tile_pool(name="psA", bufs=2, space="PSUM") as psA, \
                 tc.tile_pool(name="psG", bufs=1, space="PSUM") as psG, \
                 tc.tile_pool(name="psT2", bufs=1, space="PSUM") as psT2:
                qtT = ap.tile([128, 2, BLOC], f16)      # query, [h, b]
                ct = ap.tile([128, 2, BLOC], f32)       # 2*cell state
                att_sb = ap.tile([BLOC, E], f32)
                attT = ap.tile([128, 2, BLOC], f16)
                wTn = ap.tile([128, tot_chunks, BLOC], f16)  # exp weights
                nc.vector.memset(qtT[:], 0.0)
                nc.vector.memset(ct[:], 0.0)

                for it in range(N_SHUFFLE):
                    if it > 0:
                        # logits chunk-stationary; exp pipelined per group
                        for g in range(n_grp):
                            nch = min(8, tot_chunks - g * 8)
                            lgp = psL.tile([128, 8, BLOC], f32, tag="lgp")
                            for ci in range(nch):
                                c = g * 8 + ci
                                for kc in range(2):
                                    nc.tensor.matmul(
                                        lgp[:, ci, :],
                                        embA[:, kc, c * 128:(c + 1) * 128],
                                        qtT[:, kc, :],
                                        start=(kc == 0), stop=(kc == 1),
                                    )
                            lgs = ad.tile([128, 8, BLOC], f32, tag="lgs")
                            nc.vector.tensor_tensor(
                                out=lgs[:, :nch, :], in0=lgp[:, :nch, :],
                                in1=mask[:, g * 8: g * 8 + nch, :],
                                op=mybir.AluOpType.add,
                            )
                            nc.scalar.activation(
                                out=wTn[:, g * 8: g * 8 + nch, :],
                                in_=lgs[:, :nch, :],
                                func=mybir.ActivationFunctionType.Exp,
                            )
                        wsrc = wTn
                    else:
                        wsrc = w0T

                    # attended + S in one accumulation chain, M=16, N=257
                    att_ps = psA.tile([BLOC, E + 1], f32, tag="attps")
                    for c in range(tot_chunks):
                        nc.tensor.matmul(
                            att_ps[:, :], wsrc[:, c, :], embB[:, c, :],
                            start=(c == 0), stop=(c == tot_chunks - 1),
                        )
                    # normalize: att = att_unnorm * (1/S)
                    rS = ad.tile([BLOC, 1], f32, tag="rS")
                    nc.vector.reciprocal(rS[:], att_ps[:, E:E + 1])
                    nc.vector.tensor_scalar(
                        out=att_sb[:], in0=att_ps[:, :E], scalar1=rS[:],
                        scalar2=0.0, op0=mybir.AluOpType.mult,
                        op1=mybir.AluOpType.add,
                    )
                    # attT: [16, 256] -> [128, 2, 16]
                    pt = psT2.tile([128, 2, BLOC], f32, tag="pt")
                    for c in range(2):
                        nc.tensor.transpose(
                            pt[:, c, :], att_sb[:, c * 128:(c + 1) * 128],
                            ident[:BLOC, :BLOC],
                        )
                    nc.scalar.copy(out=attT[:], in_=pt[:])

                    # LSTM gates = Wih @ att + Whh @ qt  (+bg via act bias)
                    g_ps = psG.tile([128, 8, BLOC], f32)
                    for mc in range(8):
                        msl = slice(mc * 128, (mc + 1) * 128)
                        for kc in range(2):
                            nc.tensor.matmul(
                                g_ps[:, mc, :], wih[:, kc, msl], attT[:, kc, :],
                                start=(kc == 0), stop=False,
                            )
                        for kc in range(2):
                            nc.tensor.matmul(
                                g_ps[:, mc, :], whh[:, kc, msl],
                                qtT[:, kc, :],
                                start=False, stop=(kc == 1),
                            )
                    # gate nonlinearities via tanh only (no table swap):
                    #   sigmoid(x) = 0.5*(1 + tanh(x/2)); cell kept as 2*c.
                    ti = ad.tile([128, 2, BLOC], f32, tag="ti")
                    tf = ad.tile([128, 2, BLOC], f32, tag="tf")
                    gg = ad.tile([128, 2, BLOC], f32, tag="gg")
                    to = ad.tile([128, 2, BLOC], f32, tag="to")
                    for c in range(2):
                        nc.scalar.activation(
                            out=ti[:, c, :], in_=g_ps[:, c, :],
                            func=mybir.ActivationFunctionType.Tanh,
                            bias=bg[:, c:c + 1], scale=0.5,
                        )
                        nc.scalar.activation(
                            out=tf[:, c, :], in_=g_ps[:, 2 + c, :],
                            func=mybir.ActivationFunctionType.Tanh,
                            bias=bg[:, 2 + c:3 + c], scale=0.5,
                        )
                        nc.scalar.activation(
                            out=gg[:, c, :], in_=g_ps[:, 4 + c, :],
                            func=mybir.ActivationFunctionType.Tanh,
                            bias=bg[:, 4 + c:5 + c], scale=1.0,
                        )
                        nc.scalar.activation(
                            out=to[:, c, :], in_=g_ps[:, 6 + c, :],
                            func=mybir.ActivationFunctionType.Tanh,
                            bias=bg[:, 6 + c:7 + c], scale=0.5,
                        )
                    # ct' = f*ct + i*g  with ct stored as 2*c:
                    #   ct2' = 0.5*(1+tf)*ct2 + (1+ti)*g
                    a = ad.tile([128, 2, BLOC], f32, tag="a")
                    v = ad.tile([128, 2, BLOC], f32, tag="v")
                    nc.vector.tensor_tensor(
                        out=a[:], in0=tf[:], in1=ct[:], op=mybir.AluOpType.mult
                    )
                    nc.vector.tensor_tensor(
                        out=a[:], in0=a[:], in1=ct[:], op=mybir.AluOpType.add
                    )
                    nc.vector.tensor_tensor(
                        out=v[:], in0=ti[:], in1=gg[:], op=mybir.AluOpType.mult
                    )
                    nc.vector.tensor_tensor(
                        out=v[:], in0=v[:], in1=gg[:], op=mybir.AluOpType.add
                    )
                    nc.vector.tensor_scalar(
                        out=a[:], in0=a[:], scalar1=0.5, scalar2=0.0,
                        op0=mybir.AluOpType.mult, op1=mybir.AluOpType.add,
                    )
                    nc.vector.tensor_tensor(
                        out=ct[:], in0=a[:], in1=v[:], op=mybir.AluOpType.add
                    )
                    # h = sigmoid(o)*tanh(c) = 0.5*(1+to)*tanh(0.5*ct2)
                    th = ad.tile([128, 2, BLOC], f32, tag="th")
                    for c in range(2):
                        nc.scalar.activation(
                            out=th[:, c, :], in_=ct[:, c, :],
                            func=mybir.ActivationFunctionType.Tanh,
                            scale=0.5,
                        )
                    qt32 = ad.tile([128, 2, BLOC], f32, tag="qt32")
                    nc.vector.tensor_tensor(
                        out=qt32[:], in0=to[:], in1=th[:], op=mybir.AluOpType.mult
                    )
                    nc.vector.tensor_tensor(
                        out=qt32[:], in0=qt32[:], in1=th[:], op=mybir.AluOpType.add
                    )
                    nc.vector.tensor_scalar(
                        out=qt32[:], in0=qt32[:], scalar1=0.5, scalar2=0.0,
                        op0=mybir.AluOpType.mult, op1=mybir.AluOpType.add,
                    )
                    nc.vector.tensor_copy(qtT[:], qt32[:])
                    if it == N_SHUFFLE - 1:
                        # outputs: att (b3 added on host) and qt
                        nc.sync.dma_start(out=att_o[:], in_=att_sb[:])
                        qt_out = ap.tile([BLOC, H], f32)
                        for c in range(2):
                            ptq = psT2.tile([BLOC, 128], f32, tag="ptq")
                            nc.tensor.transpose(
                                ptq[:], qt32[:, c, :], ident[:, :]
                            )
                            nc.vector.tensor_copy(
                                qt_out[:, c * 128:(c + 1) * 128], ptq[:]
                            )
                        nc.sync.dma_start(out=qt_o[:], in_=qt_out[:])

    _split_multi_waits(nc)
    return nc


def _assign_slots(lengths):
    """LPT assignment: 16 sequences per core, balancing total tokens."""
    order = np.argsort(-lengths, kind="stable")
    sums = np.zeros(NCORES, dtype=np.int64)
    cnts = np.zeros(NCORES, dtype=np.int64)
    assign = [[] for _ in range(NCORES)]
    for b in order:
        open_cores = [c for c in range(NCORES) if cnts[c] < BLOC]
        c = min(open_cores, key=lambda c: sums[c])
        assign[c].append(int(b))
        sums[c] += int(lengths[b])
        cnts[c] += 1
    return assign, sums


def kernel(state, length, W1, b1, W2, b2, W3, b3, W_ih, W_hh, b_ih, b_hh):
    state = np.asarray(state, dtype=np.float32)
    length = np.asarray(length, dtype=np.int32)
    lengths = length.astype(np.int64)

    assign, sums = _assign_slots(lengths)
    t_common = -(-int(sums.max()) // 2048) * 2048  # multiple of 2048 (512*4)
    tot_chunks = t_common // 128

    nc = _build_module(t_common)

    # host-side weight prep (shared across cores)
    w1h = W1.T.astype(np.float16)                                # [128, 512]
    w2h = np.ascontiguousarray(
        W2.T.reshape(4, 128, H2).transpose(1, 0, 2)
    ).astype(np.float16)                                         # [128, 4, 512]
    w3h = np.ascontiguousarray(
        W3.T.reshape(4, 128, E).transpose(1, 0, 2)
    ).astype(np.float16)                                         # [128, 4, 256]
    wihh = np.ascontiguousarray(
        W_ih.T.reshape(2, 128, 4 * H).transpose(1, 0, 2)
    ).astype(np.float16)                                         # [128, 2, 1024]
    whhh = np.ascontiguousarray(
        W_hh.T.reshape(2, 128, 4 * H).transpose(1, 0, 2)
    ).astype(np.float16)
    b1h = np.ascontiguousarray(b1.reshape(4, 128).T).astype(np.float32)
    b2h = np.ascontiguousarray(b2.reshape(4, 128).T).astype(np.float32)
    bgv = (b_ih + b_hh + W_ih @ b3).astype(np.float32)
    # i, f, o gate biases pre-halved for the tanh-based sigmoid; g unchanged.
    bgv = bgv.reshape(4, H)
    bgv = np.concatenate([0.5 * bgv[0], 0.5 * bgv[1], bgv[2], 0.5 * bgv[3]])
    bgh = np.ascontiguousarray(bgv.reshape(8, 128).T).astype(np.float32)
    identh = np.eye(128, dtype=np.float32)
    identh16 = np.eye(128, dtype=np.float16)

    in_maps = []
    for c in range(NCORES):
        bidx = assign[c]  # batch index per slot
        xT = np.zeros((128, t_common), dtype=np.float16)
        maskh = np.full((128, tot_chunks, BLOC), NEG, dtype=np.float32)
        w0T = np.zeros((128, tot_chunks, BLOC), dtype=np.float16)
        off = 0
        for j in range(BLOC):
            ln = int(lengths[bidx[j]])
            xT[:, off:off + ln] = state[bidx[j], :ln, :].T
            flat = np.zeros(t_common, dtype=bool)
            flat[off:off + ln] = True
            vT = flat.reshape(tot_chunks, 128).T  # [128, tot_chunks]
            maskh[:, :, j] = np.where(vT, -C1, maskh[:, :, j])
            w0T[:, :, j] = np.where(vT, 1.0 / ln, 0.0).astype(np.float16)
            off += ln
        in_maps.append({
            "xT": xT, "w1": w1h, "w2": w2h, "w3": w3h,
            "wih": wihh, "whh": whhh, "b1": b1h, "b2": b2h, "bg": bgh,
            "mask": maskh, "w0T": w0T, "ident": identh, "ident16": identh16,
        })

    res = run_bass_kernel_spmd(nc, in_maps, list(range(NCORES)))

    out = np.zeros((B, E + H), dtype=np.float32)
    for c in range(NCORES):
        att = res.results[c]["att"] + b3[None, :].astype(np.float32)
        qt = res.results[c]["qt"]
        for j in range(BLOC):
            out[assign[c][j], :E] = att[j]
            out[assign[c][j], E:] = qt[j]
    return out


# revision 12
# speedup vs baseline: 1.0437x; 1.0437x over previous
"""Trainium2 Bass kernel for nn_CopiedSetEncoder (set encoder with recurrent
attention). Self-contained: shards batch across 8 NeuronCores, builds a
length-specialized SPMD Tile kernel, runs it, and reassembles the output.

v2 design notes (vs baseline):
- Contiguous token packing (no per-slot 128-chunk alignment): t_common drops
  ~13%. Slot ownership is encoded purely in the mask / weight tiles.
- embB (token-major embeddings) produced by PE transposes of embA instead of
  a second bank of W3 matmuls: 4096 -> 1024 PE rows per 512-token tile.
- Softmax uses unnormalized f16 weights exp(logit - C1) with C1=4 (logits on
  this problem are ~|0.2|), and a ones-column appended to embB so one matmul
  chain yields both attended and the normalizer S. Normalization happens on
  the [16, 257] result. This kills the per-iteration sum-matmul chain (68
  matmuls), reciprocal broadcast transposes, and the big normalize multiply.
- LSTM sigmoids computed as 0.5*(1+tanh(x/2)): exp/tanh/relu/copy share one
  activation table set, so the Act engine never swaps tables (saves ~1.3us
  per swap, 2 swaps/iteration in the baseline).
"""
import numpy as np

import concourse.bass as bass
import concourse.mybir as mybir
import concourse.tile as tile
from concourse.bass_utils import run_bass_kernel_spmd

B, F_, D_IN = 128, 1024, 128
H1, H2, E, H = 512, 512, 256, 256
N_SHUFFLE = 5
NCORES = 8
BLOC = B // NCORES  # 16 batches per core
NEG = -1e30
C1 = 4.0  # logit shift for max-free softmax (logits here are tiny)

f32 = mybir.dt.float32
f16 = mybir.dt.float16


def _split_multi_waits(nc):
    """HW allows at most one sync wait per instruction; hoist extras into
    standalone InstEventSemaphore carriers on the same engine."""
    cnt = 0
    for bb in nc.main_func.blocks:
        insts = bb.instructions  # live list
        i = 0
        while i < len(insts):
            ins = insts[i]
            si = ins.sync_info
            if si is not None and si.on_wait and len(si.on_wait) > 1:
                waits = list(si.on_wait)
                carriers = []
                for w in waits[:-1]:
                    cnt += 1
                    ev = mybir.InstEventSemaphore(name=f"wsplit-{cnt}")
                    ev.engine = ins.engine
                    ev.sync_info = mybir.SyncInfo(on_wait=[w], on_update=[])
                    carriers.append(ev)
                ins.sync_info = mybir.SyncInfo(
                    on_wait=[waits[-1]], on_update=list(si.on_update)
                )
                for j, ev in enumerate(carriers):
                    insts.insert(i + j, ev)
                    nc.register_instruction(ev, overwrite=True)
                i += len(carriers)
            i += 1
    return cnt


def _build_module(t_common):
    """One SPMD program for all cores. Tokens packed contiguously per core;
    slot membership is data (mask/w0T), not structure."""
    nc = bass.Bass()
    n_tiles = t_common // 512
    tot_chunks = t_common // 128

    # ---- inputs ----
    xT_e = nc.declare_dram_parameter("xT", [128, t_common], f16, isOutput=False)
    w1_e = nc.declare_dram_parameter("w1", [128, H1], f16, isOutput=False)
    w2_e = nc.declare_dram_parameter("w2", [128, 4, H2], f16, isOutput=False)
    w3_e = nc.declare_dram_parameter("w3", [128, 4, E], f16, isOutput=False)
    wih_e = nc.declare_dram_parameter("wih", [128, 2, 4 * H], f16, isOutput=False)
    whh_e = nc.declare_dram_parameter("whh", [128, 2, 4 * H], f16, isOutput=False)
    b1_e = nc.declare_dram_parameter("b1", [128, 4], f32, isOutput=False)
    b2_e = nc.declare_dram_parameter("b2", [128, 4], f32, isOutput=False)
    bg_e = nc.declare_dram_parameter("bg", [128, 8], f32, isOutput=False)
    mask_e = nc.declare_dram_parameter(
        "mask", [128, tot_chunks, BLOC], f32, isOutput=False
    )
    w0T_e = nc.declare_dram_parameter(
        "w0T", [128, tot_chunks, BLOC], f16, isOutput=False
    )
    ident_e = nc.declare_dram_parameter("ident", [128, 128], f32, isOutput=False)
    ident16_e = nc.declare_dram_parameter("ident16", [128, 128], f16, isOutput=False)
    att_o = nc.declare_dram_parameter("att", [BLOC, E], f32, isOutput=True)
    qt_o = nc.declare_dram_parameter("qt", [BLOC, H], f32, isOutput=True)

    with tile.TileContext(nc) as tc:
        with tc.tile_pool(name="big", bufs=1) as big, \
             tc.tile_pool(name="wp", bufs=1) as wp:
            # resident tensors
            xT = big.tile([128, t_common], f16)
            embA = big.tile([128, 2, t_common], f16)
            embB = big.tile([128, tot_chunks, E + 1], f16)
            w1 = wp.tile([128, H1], f16)
            w2 = wp.tile([128, 4, H2], f16)
            w3 = wp.tile([128, 4, E], f16)
            wih = wp.tile([128, 2, 4 * H], f16)
            whh = wp.tile([128, 2, 4 * H], f16)
            b1 = wp.tile([128, 4], f32)
            b2 = wp.tile([128, 4], f32)
            bg = wp.tile([128, 8], f32)
            mask = wp.tile([128, tot_chunks, BLOC], f32)
            w0T = wp.tile([128, tot_chunks, BLOC], f16)
            ident = wp.tile([128, 128], f32)
            ident16 = wp.tile([128, 128], f16)
            # DMA order: what tile 0 needs first, bulk xT interleaved, then
            # attention-phase-only tensors last.
            xq = t_common // 4  # t_common is a multiple of 2048
            nc.sync.dma_start(out=xT[:, 0:xq], in_=xT_e[:, 0:xq])
            for dst, src in [(w1, w1_e), (b1, b1_e), (w2, w2_e), (b2, b2_e)]:
                nc.sync.dma_start(out=dst[:], in_=src[:])
            nc.sync.dma_start(out=xT[:, xq:2 * xq], in_=xT_e[:, xq:2 * xq])
            for dst, src in [(w3, w3_e), (ident16, ident16_e)]:
                nc.sync.dma_start(out=dst[:], in_=src[:])
            nc.sync.dma_start(out=xT[:, 2 * xq:3 * xq], in_=xT_e[:, 2 * xq:3 * xq])
            nc.sync.dma_start(out=xT[:, 3 * xq:], in_=xT_e[:, 3 * xq:])
            for dst, src in [
                (wih, wih_e), (whh, whh_e), (bg, bg_e), (w0T, w0T_e),
                (mask, mask_e), (ident, ident_e),
            ]:
                nc.sync.dma_start(out=dst[:], in_=src[:])
            # ones column of embB (col E), once
            nc.vector.memset(embB[:, :, E:E + 1], 1.0)

            # ---- phase 1: MLP over 512-token tiles; embB via PE transpose ----
            with tc.tile_pool(name="mlp", bufs=3) as mp, \
                 tc.tile_pool(name="ps1", bufs=2, space="PSUM") as ps1, \
                 tc.tile_pool(name="ps2", bufs=2, space="PSUM") as ps2, \
                 tc.tile_pool(name="ps3", bufs=2, space="PSUM") as ps3, \
                 tc.tile_pool(name="psT", bufs=2, space="PSUM") as psT:
                for t in range(n_tiles):
                    sl = slice(t * 512, (t + 1) * 512)
                    h1t = mp.tile([128, 4, 512], f16, tag="h1")
                    for mc in range(4):
                        p = ps1.tile([128, 512], f32, tag="pA")
                        nc.tensor.matmul(
                            p[:], w1[:, mc * 128:(mc + 1) * 128], xT[:, sl],
                            start=True, stop=True,
                        )
                        if mc % 2 == 0:
                            nc.scalar.activation(
                                out=h1t[:, mc, :], in_=p[:],
                                func=mybir.ActivationFunctionType.Relu,
                                bias=b1[:, mc:mc + 1], scale=1.0,
                            )
                        else:
                            nc.vector.tensor_scalar(
                                out=h1t[:, mc, :], in0=p[:], scalar1=b1[:, mc:mc + 1],
                                scalar2=0.0, op0=mybir.AluOpType.add,
                                op1=mybir.AluOpType.max,
                            )
                    h2t = mp.tile([128, 4, 512], f16, tag="h2")
                    for mc in range(4):
                        p = ps2.tile([128, 512], f32, tag="pB")
                        for kc in range(4):
                            nc.tensor.matmul(
                                p[:], w2[:, kc, mc * 128:(mc + 1) * 128],
                                h1t[:, kc, :], start=(kc == 0), stop=(kc == 3),
                            )
                        if mc % 2 == 0:
                            nc.scalar.activation(
                                out=h2t[:, mc, :], in_=p[:],
                                func=mybir.ActivationFunctionType.Relu,
                                bias=b2[:, mc:mc + 1], scale=1.0,
                            )
                        else:
                            nc.vector.tensor_scalar(
                                out=h2t[:, mc, :], in0=p[:], scalar1=b2[:, mc:mc + 1],
                                scalar2=0.0, op0=mybir.AluOpType.add,
                                op1=mybir.AluOpType.max,
                            )
                    # embA: [e-chunk partitions, tokens]
                    for mc in range(2):
                        p = ps3.tile([128, 512], f32, tag="pC")
                        for kc in range(4):
                            nc.tensor.matmul(
                                p[:], w3[:, kc, mc * 128:(mc + 1) * 128],
                                h2t[:, kc, :], start=(kc == 0), stop=(kc == 3),
                            )
                        if mc == 0:
                            nc.scalar.copy(out=embA[:, mc, sl], in_=p[:])
                        else:
                            nc.vector.tensor_copy(embA[:, mc, sl], p[:])
                    # embB: transpose embA 128-token blocks -> [tok, E]
                    for s in range(4):
                        ch = t * 4 + s
                        tsl = slice(t * 512 + s * 128, t * 512 + (s + 1) * 128)
                        pt = psT.tile([128, 2, 128], f16, tag="pT")
                        for kc in range(2):
                            nc.tensor.transpose(
                                pt[:, kc, :], embA[:, kc, tsl], ident16[:, :]
                            )
                        if s % 2 == 0:
                            nc.scalar.copy(out=embB[:, ch, :E], in_=pt[:])
                        else:
                            nc.vector.tensor_copy(embB[:, ch, :E], pt[:])

            # ---- phase 2: recurrent attention ----
            n_grp = (tot_chunks + 7) // 8
            with tc.tile_pool(name="att", bufs=1) as ap, \
                 tc.tile_pool(name="attd", bufs=2) as ad, \
                 tc.tile_pool(name="psL", bufs=2, space="PSUM") as psL, \
                 tc.tile_pool(name="psA", bufs=2, space="PSUM") as psA, \
                 tc.tile_pool(name="psG", bufs=1, space="PSUM") as psG, \
                 tc.tile_pool(name="psT2", bufs=1, space="PSUM") as psT2:
                qtT = ap.tile([128, 2, BLOC], f16)      # query, [h, b]
                ct = ap.tile([128, 2, BLOC], f32)       # 2*cell state
                att_sb = ap.tile([BLOC, E], f32)
                attT = ap.tile([128, 2, BLOC], f16)
                wTn = ap.tile([128, tot_chunks, BLOC], f16)  # exp weights
                nc.vector.memset(qtT[:], 0.0)
                nc.vector.memset(ct[:], 0.0)

                for it in range(N_SHUFFLE):
                    if it > 0:
                        # logits chunk-stationary; exp pipelined per group
                        for g in range(n_grp):
                            nch = min(8, tot_chunks - g * 8)
                            lgp = psL.tile([128, 8, BLOC], f32, tag="lgp")
                            for ci in range(nch):
                                c = g * 8 + ci
                                for kc in range(2):
                                    nc.tensor.matmul(
                                        lgp[:, ci, :],
                                        embA[:, kc, c * 128:(c + 1) * 128],
                                        qtT[:, kc, :],
                                        start=(kc == 0), stop=(kc == 1),
                                    )
                            lgs = ad.tile([128, 8, BLOC], f32, tag="lgs")
                            nc.vector.tensor_tensor(
                                out=lgs[:, :nch, :], in0=lgp[:, :nch, :],
                                in1=mask[:, g * 8: g * 8 + nch, :],
                                op=mybir.AluOpType.add,
                            )
                            nc.scalar.activation(
                                out=wTn[:, g * 8: g * 8 + nch, :],
                                in_=lgs[:, :nch, :],
                                func=mybir.ActivationFunctionType.Exp,
                            )
                        wsrc = wTn
                    else:
                        wsrc = w0T

                    # attended + S in one accumulation chain, M=16, N=257
                    att_ps = psA.tile([BLOC, E + 1], f32, tag="attps")
                    for c in range(tot_chunks):
                        nc.tensor.matmul(
                            att_ps[:, :], wsrc[:, c, :], embB[:, c, :],
                            start=(c == 0), stop=(c == tot_chunks - 1),
                        )
                    # normalize: att = att_unnorm * (1/S)
                    rS = ad.tile([BLOC, 1], f32, tag="rS")
                    nc.vector.reciprocal(rS[:], att_ps[:, E:E + 1])
                    nc.vector.tensor_scalar(
                        out=att_sb[:], in0=att_ps[:, :E], scalar1=rS[:],
                        scalar2=0.0, op0=mybir.AluOpType.mult,
                        op1=mybir.AluOpType.add,
                    )
                    # attT: [16, 256] -> [128, 2, 16]
                    pt = psT2.tile([128, 2, BLOC], f32, tag="pt")
                    for c in range(2):
                        nc.tensor.transpose(
                            pt[:, c, :], att_sb[:, c * 128:(c + 1) * 128],
                            ident[:BLOC, :BLOC],
                        )
                    nc.scalar.copy(out=attT[:], in_=pt[:])

                    # LSTM gates = Wih @ att + Whh @ qt  (+bg via act bias)
                    g_ps = psG.tile([128, 8, BLOC], f32, tag="gps")
                    for mc in range(8):
                        msl = slice(mc * 128, (mc + 1) * 128)
                        for kc in range(2):
                            nc.tensor.matmul(
                                g_ps[:, mc, :], wih[:, kc, msl], attT[:, kc, :],
                                start=(kc == 0), stop=False,
                            )
                        for kc in range(2):
                            nc.tensor.matmul(
                                g_ps[:, mc, :], whh[:, kc, msl],
                                qtT[:, kc, :],
                                start=False, stop=(kc == 1),
                            )
                    # gate nonlinearities via tanh only (no table swap):
                    #   sigmoid(x) = 0.5*(1 + tanh(x/2)); cell kept as 2*c.
                    ti = ad.tile([128, 2, BLOC], f32, tag="ti")
                    tf = ad.tile([128, 2, BLOC], f32, tag="tf")
                    gg = ad.tile([128, 2, BLOC], f32, tag="gg")
                    to = ad.tile([128, 2, BLOC], f32, tag="to")
                    for c in range(2):
                        nc.scalar.activation(
                            out=ti[:, c, :], in_=g_ps[:, c, :],
                            func=mybir.ActivationFunctionType.Tanh,
                            bias=bg[:, c:c + 1], scale=0.5,
                        )
                        nc.scalar.activation(
                            out=tf[:, c, :], in_=g_ps[:, 2 + c, :],
                            func=mybir.ActivationFunctionType.Tanh,
                            bias=bg[:, 2 + c:3 + c], scale=0.5,
                        )
                        nc.scalar.activation(
                            out=gg[:, c, :], in_=g_ps[:, 4 + c, :],
                            func=mybir.ActivationFunctionType.Tanh,
                            bias=bg[:, 4 + c:5 + c], scale=1.0,
                        )
                        nc.scalar.activation(
                            out=to[:, c, :], in_=g_ps[:, 6 + c, :],
                            func=mybir.ActivationFunctionType.Tanh,
                            bias=bg[:, 6 + c:7 + c], scale=0.5,
                        )
                    # ct' = f*ct + i*g  with ct stored as 2*c:
                    #   ct2' = 0.5*(1+tf)*ct2 + (1+ti)*g
                    a = ad.tile([128, 2, BLOC], f32, tag="a")
                    v = ad.tile([128, 2, BLOC], f32, tag="v")
                    nc.vector.tensor_tensor(
                        out=a[:], in0=tf[:], in1=ct[:], op=mybir.AluOpType.mult
                    )
                    nc.vector.tensor_tensor(
                        out=a[:], in0=a[:], in1=ct[:], op=mybir.AluOpType.add
                    )
                    nc.vector.tensor_tensor(
                        out=v[:], in0=ti[:], in1=gg[:], op=mybir.AluOpType.mult
                    )
                    nc.vector.tensor_tensor(
                        out=v[:], in0=v[:], in1=gg[:], op=mybir.AluOpType.add
                    )
                    nc.vector.tensor_scalar(
                        out=a[:], in0=a[:], scalar1=0.5, scalar2=0.0,
                        op0=mybir.AluOpType.mult, op1=mybir.AluOpType.add,
                    )
                    nc.vector.tensor_tensor(
                        out=ct[:], in0=a[:], in1=v[:], op=mybir.AluOpType.add
                    )
                    # h = sigmoid(o)*tanh(c) = 0.5*(1+to)*tanh(0.5*ct2)
                    th = ad.tile([128, 2, BLOC], f32, tag="th")
                    for c in range(2):
                        nc.scalar.activation(
                            out=th[:, c, :], in_=ct[:, c, :],
                            func=mybir.ActivationFunctionType.Tanh,
                            scale=0.5,
                        )
                    qt32 = ad.tile([128, 2, BLOC], f32, tag="qt32")
                    nc.vector.tensor_tensor(
                        out=qt32[:], in0=to[:], in1=th[:], op=mybir.AluOpType.mult
                    )
                    nc.vector.tensor_tensor(
                        out=qt32[:], in0=qt32[:], in1=th[:], op=mybir.AluOpType.add
                    )
                    nc.vector.tensor_scalar(
                        out=qt32[:], in0=qt32[:], scalar1=0.5, scalar2=0.0,
                        op0=mybir.AluOpType.mult, op1=mybir.AluOpType.add,
                    )
                    nc.vector.tensor_copy(qtT[:], qt32[:])
                    if it == N_SHUFFLE - 1:
                        # outputs: att (b3 added on host) and qt
                        nc.sync.dma_start(out=att_o[:], in_=att_sb[:])
                        qt_out = ap.tile([BLOC, H], f32)
                        for c in range(2):
                            ptq = psT2.tile([BLOC, 128], f32, tag="ptq")
                            nc.tensor.transpose(
                                ptq[:], qt32[:, c, :], ident[:, :]
                            )
                            nc.vector.tensor_copy(
                                qt_out[:, c * 128:(c + 1) * 128], ptq[:]
                            )
                        nc.sync.dma_start(out=qt_o[:], in_=qt_out[:])

    _split_multi_waits(nc)
    return nc


def _assign_slots(lengths):
    """LPT assignment: 16 sequences per core, balancing total tokens."""
    order = np.argsort(-lengths, kind="stable")
    sums = np.zeros(NCORES, dtype=np.int64)
    cnts = np.zeros(NCORES, dtype=np.int64)
    assign = [[] for _ in range(NCORES)]
    for b in order:
        open_cores = [c for c in range(NCORES) if cnts[c] < BLOC]
        c = min(open_cores, key=lambda c: sums[c])
        assign[c].append(int(b))
        sums[c] += int(lengths[b])
        cnts[c] += 1
    return assign, sums


def kernel(state, length, W1, b1, W2, b2, W3, b3, W_ih, W_hh, b_ih, b_hh):
    state = np.asarray(state, dtype=np.float32)
    length = np.asarray(length, dtype=np.int32)
    lengths = length.astype(np.int64)

    assign, sums = _assign_slots(lengths)
    t_common = -(-int(sums.max()) // 2048) * 2048  # multiple of 2048 (512*4)
    tot_chunks = t_common // 128

    nc = _build_module(t_common)

    # host-side weight prep (shared across cores)
    w1h = W1.T.astype(np.float16)                                # [128, 512]
    w2h = np.ascontiguousarray(
        W2.T.reshape(4, 128, H2).transpose(1, 0, 2)
    ).astype(np.float16)                                         # [128, 4, 512]
    w3h = np.ascontiguousarray(
        W3.T.reshape(4, 128, E).transpose(1, 0, 2)
    ).astype(np.float16)                                         # [128, 4, 256]
    wihh = np.ascontiguousarray(
        W_ih.T.reshape(2, 128, 4 * H).transpose(1, 0, 2)
    ).astype(np.float16)                                         # [128, 2, 1024]
    whhh = np.ascontiguousarray(
        W_hh.T.reshape(2, 128, 4 * H).transpose(1, 0, 2)
    ).astype(np.float16)
    b1h = np.ascontiguousarray(b1.reshape(4, 128).T).astype(np.float32)
    b2h = np.ascontiguousarray(b2.reshape(4, 128).T).astype(np.float32)
    bgv = (b_ih + b_hh + W_ih @ b3).astype(np.float32)
    # i, f, o gate biases pre-halved for the tanh-based sigmoid; g unchanged.
    bgv = bgv.reshape(4, H)
    bgv = np.concatenate([0.5 * bgv[0], 0.5 * bgv[1], bgv[2], 0.5 * bgv[3]])
    bgh = np.ascontiguousarray(bgv.reshape(8, 128).T).astype(np.float32)
    identh = np.eye(128, dtype=np.float32)
    identh16 = np.eye(128, dtype=np.float16)

    in_maps = []
    for c in range(NCORES):
        bidx = assign[c]  # batch index per slot
        xT = np.zeros((128, t_common), dtype=np.float16)
        maskh = np.full((128, tot_chunks, BLOC), NEG, dtype=np.float32)
        w0T = np.zeros((128, tot_chunks, BLOC), dtype=np.float16)
        off = 0
        for j in range(BLOC):
            ln = int(lengths[bidx[j]])
            xT[:, off:off + ln] = state[bidx[j], :ln, :].T
            flat = np.zeros(t_common, dtype=bool)
            flat[off:off + ln] = True
            vT = flat.reshape(tot_chunks, 128).T  # [128, tot_chunks]
            maskh[:, :, j] = np.where(vT, -C1, maskh[:, :, j])
            w0T[:, :, j] = np.where(vT, 1.0 / ln, 0.0).astype(np.float16)
            off += ln
        in_maps.append({
            "xT": xT, "w1": w1h, "w2": w2h, "w3": w3h,
            "wih": wihh, "whh": whhh, "b1": b1h, "b2": b2h, "bg": bgh,
            "mask": maskh, "w0T": w0T, "ident": identh, "ident16": identh16,
        })

    res = run_bass_kernel_spmd(nc, in_maps, list(range(NCORES)))

    out = np.zeros((B, E + H), dtype=np.float32)
    for c in range(NCORES):
        att = res.results[c]["att"] + b3[None, :].astype(np.float32)
        qt = res.results[c]["qt"]
        for j in range(BLOC):
            out[assign[c][j], :E] = att[j]
            out[assign[c][j], E:] = qt[j]
    return out
